# revision 1
# baseline (speedup 1.0000x reference)
"""Trainium2 Bass kernel for nn_DAGLayer (gnn_message_passing).

Problem: out buffer holds L leaf columns followed by M computed nodes.
Node i gathers P=8 parent columns (each [N, C]) from the buffer, applies a
per-node dense map y = einsum('ncp,ocp->no', g, W[i]) + b[i], and appends y.

Strategy (8 NeuronCores, one SPMD program):
  - Host computes DAG levels from `parents`; nodes of one level are
    independent, so each level is one parallel "round".
  - Within a round the nodes are dealt round-robin to the 8 cores
    (node-parallel; per-node weights live only on the owning core, so the
    1 GB weight tensor is sharded 8x - the kernel is weight-DMA bound).
  - Every core keeps a replicated history buffer `hbuf` in DRAM holding all
    node outputs as [slot, c, n] fp16 blocks; after each round an AllGather
    concatenates the 8 cores' new outputs into everyone's hbuf.
  - Parent gathers are plain contiguous DMAs whose row offset is a runtime
    register loaded from a per-core offset table (single SPMD program, the
    per-core differences live entirely in input data). c-major slot layout
    puts the contraction dim on partitions directly - no transposes.
  - Per node: 16 gathered [128, 32] tiles, 32 accumulating fp16 matmuls
    (stationary weights [128k x 128o] with fast-weight-load, moving [128,32])
    into 2 PSUM tiles [128o, 32n], bias added by the psum->sbuf activation
    copy (bias is per-partition in this orientation).
  - Emission order overlaps each AllGather with the next round's weight
    DMAs and with gathers/matmuls of taps whose parents are >= 2 rounds old
    (~95% of taps), so only "fresh" taps wait on the collective.

Compute is fp16 with fp32 PSUM accumulation (the node outputs returned to
the host stay fp32). The kernel is self-contained; shapes and the schedule
are derived from the inputs at run time.
"""

import os

import numpy as np

os.environ.setdefault("NEURON_COMPILE_CACHE_URL", "/root/neuron_cache")

NCORES = 8

_BUILD_CACHE = {}


def _compute_levels(parents, L, M):
    lvl = np.zeros(L + M, np.int64)
    pare = np.asarray(parents, np.int64)
    for i in range(M):
        lvl[L + i] = 1 + lvl[pare[i]].max()
    nlev = int(lvl[L:].max()) if M else 0
    level_nodes = [np.nonzero(lvl[L:] == d)[0] for d in range(1, nlev + 1)]
    return level_nodes, lvl


def _build_bass(L, s_list, S, total_slots, old_taps):
    """old_taps[s] = list of kk in 0..15 whose parent data is >= 2 rounds old
    (may be gathered before the previous round's AllGather)."""
    import concourse.bacc as bacc
    import concourse.bass as bass
    import concourse.mybir as mybir
    import concourse.tile as tile

    f16 = mybir.dt.float16
    f32 = mybir.dt.float32
    i32 = mybir.dt.int32

    nc = bacc.Bacc(num_devices=NCORES, num_swdge_queues=4)
    # history slots are stored [c%128 (row), (c//128, n) (64 cols)] fp16 so a
    # parent gather is a single plain 2D DMA of 128 contiguous rows.
    HROWS = total_slots * 128

    wbuf = nc.dram_tensor("wbuf", [S, 128, 16, 2, 128], f16, kind="ExternalInput")
    xt = nc.dram_tensor("xt", [L * 128, 64], f16, kind="ExternalInput")
    bbuf = nc.dram_tensor("bbuf", [128, 2 * S], f32, kind="ExternalInput")
    gidx = nc.dram_tensor("gidx", [1, 8 * S], i32, kind="ExternalInput")
    yout = nc.dram_tensor("yout", [S * 128, 64], f16, kind="ExternalOutput")
    hbuf = nc.dram_tensor("hbuf", [HROWS, 64], f16, addr_space="Shared")
    agin = nc.dram_tensor("agin", [S * 128, 64], f16)
    rg = [list(range(NCORES))]

    with tile.TileContext(nc) as tc:
        with (
            tc.tile_pool(name="const", bufs=1) as constp,
            tc.tile_pool(name="w", bufs=8) as wp,
            tc.tile_pool(name="g", bufs=6) as gp,
            tc.tile_pool(name="y", bufs=8) as yp,
            tc.tile_pool(name="py", bufs=4, space="PSUM") as pyp,
        ):
            b_sb = constp.tile([128, 2 * S], f32)
            nc.sync.dma_start(b_sb[:], bbuf[:])
            gidx_sb = constp.tile([1, 8 * S], i32)
            nc.sync.dma_start(gidx_sb[:], gidx[:])
            # leaves into the shared history buffer
            nc.gpsimd.dma_start(hbuf[0 : L * 128, :], xt[:])

            # one plain 2D gather DMA per tap (128 contiguous rows of hbuf),
            # issue cost spread across the three DMA-capable sequencers.
            tap_groups = [
                (nc.sync, (0, 1, 2, 3)),
                (nc.scalar, (4, 5, 6, 7)),
            ]

            def emit_gathers(s, g_all, taps):
                tapset = set(taps)
                for eng, etaps in tap_groups:
                    use = [t for t in etaps if t in tapset]
                    if not use:
                        continue
                    lo, hi = use[0], use[-1]
                    cols = gidx_sb[0:1, 8 * s + lo : 8 * s + hi + 1]
                    _, vals = nc.values_load_multi_w_load_instructions(
                        cols,
                        engines=[eng.engine],
                        min_val=0,
                        max_val=HROWS - 128,
                        skip_runtime_bounds_check=True,
                    )
                    for t in use:
                        eng.dma_start(
                            g_all[:, t, :], hbuf[bass.ds(vals[t - lo], 128), :]
                        )

            off = 0
            pend = []  # nodes of the current round awaiting post-AG work
            pend_meta = None
            for r, s_r in enumerate(s_list):
                # phase A of round r: weight DMAs + old-tap gathers
                # (emitted BEFORE the previous round's AllGather so they only
                # depend on older AG writes via program order)
                cur = []
                for m in range(s_r):
                    s = off + m
                    w_t = wp.tile([128, 16, 2, 128], f16, tag="w")
                    nc.gpsimd.dma_start(w_t[:], wbuf[s])
                    g_all = gp.tile([128, 8, 64], f16, tag="g")
                    old = old_taps[s]
                    emit_gathers(s, g_all, old)
                    cur.append((s, w_t, g_all, old))

                # AllGather of the previous round
                if pend:
                    ps_r, poff = pend_meta
                    gbase = L + 8 * poff
                    nc.gpsimd.collective_compute(
                        "AllGather",
                        mybir.AluOpType.bypass,
                        replica_groups=rg,
                        ins=[agin[poff * 128 : (poff + ps_r) * 128, :]],
                        outs=[hbuf[gbase * 128 : (gbase + 8 * ps_r) * 128, :]],
                    )
                    pend = []

                # phase B of round r: fresh gathers, matmuls, bias, outputs
                for s, w_t, g_all, old in cur:
                    fresh = [t for t in range(8) if t not in old]
                    emit_gathers(s, g_all, fresh)
                    th_order = [(t, h) for t in list(old) + fresh for h in range(2)]
                    pys = [
                        pyp.tile([128, 32], f32, tag="py", name=f"py{oh}")
                        for oh in range(2)
                    ]
                    for i, (t, h) in enumerate(th_order):
                        for oh in range(2):
                            nc.tensor.matmul(
                                pys[oh][:],
                                w_t[:, 2 * t + h, oh, :],
                                g_all[:, t, h * 32 : (h + 1) * 32],
                                start=(i == 0),
                                stop=(i == 15),
                            )
                    y16 = yp.tile([128, 2, 32], f16, tag="y16")
                    for oh in range(2):
                        bias = b_sb[:, 2 * s + oh : 2 * s + oh + 1]
                        nc.scalar.activation(
                            y16[:, oh, :], pys[oh][:],
                            mybir.ActivationFunctionType.Identity, bias=bias,
                        )
                    nc.sync.dma_start(agin[s * 128 : (s + 1) * 128, :], y16[:])
                pend = cur
                pend_meta = (s_r, off)
                off += s_r

            # single flush of all computed node outputs
            nc.sync.dma_start(yout[:], agin[:])
    nc.compile()
    return nc


def kernel(x, W, b, parents):
    from concourse.bass_utils import run_bass_kernel_spmd

    x = np.ascontiguousarray(np.asarray(x), dtype=np.float32)
    W = np.ascontiguousarray(np.asarray(W), dtype=np.float32)
    b = np.ascontiguousarray(np.asarray(b), dtype=np.float32)
    parents = np.asarray(parents).astype(np.int64)

    N, C, L = x.shape
    M, O, C2, P = W.shape
    assert (N, C, O, C2, P) == (32, 256, 256, 256, 8), "kernel hardcodes these dims"

    level_nodes, lvl = _compute_levels(parents, L, M)
    s_list = [(len(nodes) + NCORES - 1) // NCORES for nodes in level_nodes]
    S = sum(s_list)
    total_slots = L + 8 * S

    # slot assignment: round r occupies global slots [L+8*off_r, L+8*(off_r+s_r))
    # in AllGather rank-major order; core q's m-th slot of round r holds the
    # (q + 8*m)-th node of the level.
    slot_of = np.full(L + M, -1, np.int64)
    slot_of[:L] = np.arange(L)
    node_of_coreslot = np.full((NCORES, S), -1, np.int64)
    round_of_coreslot = np.zeros(S, np.int64)
    off = 0
    for r, nodes in enumerate(level_nodes):
        s_r = s_list[r]
        round_of_coreslot[off : off + s_r] = r
        for j, node in enumerate(nodes):
            q, m = j % NCORES, j // NCORES
            slot_of[L + node] = L + 8 * off + q * s_r + m
            node_of_coreslot[q, off + m] = node
        off += s_r
    assert (slot_of >= 0).all()

    # weight relayout: [M, o, c, p] -> [M, 128(part), 16(ktile), 2(ohalf), 128(o)]
    # with k = tap*256 + c, partition = k % 128, ktile = k // 128.
    W4 = (
        W.transpose(0, 3, 2, 1)
        .reshape(M, 16, 128, 2, 128)
        .transpose(0, 2, 1, 3, 4)
        .astype(np.float16)
    )
    # leaf slot layout [c%128, (c//128, n)]: rows of 64 fp16
    xt_host = np.ascontiguousarray(
        x.transpose(2, 1, 0)
        .reshape(L, 2, 128, 32)
        .transpose(0, 2, 1, 3)
        .reshape(L * 128, 64)
        .astype(np.float16)
    )

    # old_taps[s]: tap indices whose parent was computed >= 2 rounds before
    # the slot's round (or is a leaf) on EVERY core - those gathers may be
    # emitted before the previous round's AllGather. The program structure
    # must be identical across cores, hence the intersection.
    old_taps = []
    for s in range(S):
        r = round_of_coreslot[s]
        taps = []
        for tap in range(P):
            ok = True
            for q in range(NCORES):
                node = node_of_coreslot[q, s]
                if node < 0:
                    continue
                par = parents[node][tap]
                if par >= L and lvl[par] >= r:  # parent round is lvl-1
                    ok = False
                    break
            if ok:
                taps.append(tap)
        old_taps.append(taps)

    narange = np.arange(32, dtype=np.int64)
    in_maps = []
    for q in range(NCORES):
        nodes_q = node_of_coreslot[q]
        valid = nodes_q >= 0
        Wq = np.zeros((S, 128, 16, 2, 128), np.float16)
        Wq[valid] = W4[nodes_q[valid]]
        bq = np.zeros((S, 2, 128), np.float32)
        bq[valid] = b[nodes_q[valid]].reshape(-1, 2, 128)
        # b_sb layout [128, 2S]: [o_local, (s, oh)]
        bq2 = np.ascontiguousarray(bq.transpose(2, 0, 1).reshape(128, 2 * S))
        gq = np.zeros((1, 8 * S), np.int32)
        for s in range(S):
            node = nodes_q[s]
            par = parents[node] if node >= 0 else np.zeros(P, np.int64)
            pslots = slot_of[par]
            for tap in range(P):
                gq[0, 8 * s + tap] = pslots[tap] * 128
        in_maps.append({"wbuf": Wq, "xt": xt_host, "bbuf": bq2, "gidx": gq})

    key = (L, tuple(s_list), tuple(tuple(t) for t in old_taps))
    if key not in _BUILD_CACHE:
        import time as _time

        _t0 = _time.time()
        _BUILD_CACHE[key] = _build_bass(L, s_list, S, total_slots, old_taps)
        print(f"[kernel] bass build took {_time.time() - _t0:.1f}s", flush=True)
    nc = _BUILD_CACHE[key]

    global LAST_RUN
    LAST_RUN = (nc, in_maps)

    results = run_bass_kernel_spmd(nc, in_maps, core_ids=list(range(NCORES))).results

    out = np.zeros((N, C, L + M), np.float32)
    out[:, :, :L] = x
    for q in range(NCORES):
        # yout rows are [slot, c%128] x [c//128, n]
        yq = (
            np.asarray(results[q]["yout"])
            .astype(np.float32)
            .reshape(S, 128, 2, 32)
            .transpose(0, 3, 2, 1)
            .reshape(S, 32, 256)
        )
        for s in range(S):
            node = node_of_coreslot[q, s]
            if node >= 0:
                out[:, :, L + node] = yq[s]
    return out



# revision 7
# speedup vs baseline: 1.0910x; 1.0910x over previous
"""Trainium2 Bass kernel for nn_DAGLayer (gnn_message_passing).

Problem: out buffer holds L leaf columns followed by M computed nodes.
Node i gathers P=8 parent columns (each [N, C]) from the buffer, applies a
per-node dense map y = einsum('ncp,ocp->no', g, W[i]) + b[i], and appends y.

Strategy (8 NeuronCores, one SPMD program):
  - Host schedules nodes into rounds with a LOCALITY rule: a node may depend
    on rounds newer than the last completed AllGather only via parents that
    were computed on the SAME core.  Those "window" parents are gathered from
    a core-local DRAM staging buffer (`own`), so the AllGather is never on
    the round-to-round critical chain - it gets a full AG-window of slack.
  - AllGathers are batched: one AG per ~AG_T slots (covering several rounds).
  - Weights are stored as fp8 e3m4 scaled by 512 (uniform-distributed W fits
    4 mantissa bits; measured DAG rel err ~8.5e-3 vs fp32).  All stored
    activations (leaves and node outputs) are at 1/512 scale so the fp8
    scaling cancels exactly; the host rescales the output by 512.
  - Per slot: 8 gather DMAs (positions [0,a) from `own`, [a,8) from hbuf;
    taps permuted per-core on the host so the split is SPMD-uniform), then
    32 accumulating matmuls (hbuf/old taps FIRST, window/fresh taps LAST so
    only the fresh taps sit on the critical chain), bias+1/512 applied by
    the psum->sbuf activation, y written to `own` (gpsimd; same queue as the
    AGs, so AG inputs are ordered after y writes for free).
  - Engine split: weight DMAs alternate sync/scalar (HWDGE); gathers are
    positions 0-3 sync / 4-7 scalar; activations scalar; leaf init + y
    writes + AllGathers gpsimd; matmuls tensor.

Compute is e3m4[weights] x fp16[activations] with fp32 PSUM accumulation.
The kernel is self-contained; the schedule is derived from the inputs at
run time on the host.
"""

import os

import numpy as np

os.environ.setdefault("NEURON_COMPILE_CACHE_URL", "/root/neuron_cache")

NCORES = 8
AG_T = 5          # AllGather threshold: place an AG top once >= AG_T slots pending
COV_LAG = 1       # AG at top t is readable from round t+COV_LAG
W_SCALE = 512.0   # weight scale for e3m4; activations stored at 1/W_SCALE
W_DTYPE = "f8e3"  # "f8e3" or "f16"
A_MAX = 4         # own-positions per slot capped so positions [0,4) are sync

_BUILD_CACHE = {}


# ---------------------------------------------------------------- scheduler
def _compute_schedule(parents, L, M):
    """Rounds of k_r slots per core; AG tops; per-slot own-position count a_s.

    Constraints for node v at (round r, core c):
      - parents with round > cov(r) must be on core c (read from `own`);
        cov(r) = last AG top <= r-1 covers rounds <= top-1.
      - per slot s the tap split [0,a_s)=own / [a_s,8)=hbuf is shared across
        cores: a_s >= #window-parents and the node must have >= a_s parents
        that are leaves or core-local (own-readable), and its remote covered
        parents must fit in the 8-a_s hbuf positions.
    """
    NC = NCORES
    children = [[] for _ in range(L + M)]
    for i in range(M):
        for p in set(parents[i].tolist()):
            children[p].append(L + i)
    height = np.zeros(L + M, np.int64)
    for v in range(L + M - 1, L - 1, -1):
        for ch in children[v]:
            height[v] = max(height[v], height[ch] + 1)
    round_of = np.full(L + M, -1, np.int64)
    core_of = np.full(L + M, -1, np.int64)
    round_of[:L] = -(10**9)
    unsched = set(range(L, L + M))
    core_lock = {}
    rounds = []
    tops = []            # round indices with an AG at their top
    slots_done = 0       # slots of completed rounds
    covered_slots = 0    # slots covered by emitted AGs
    cov_round = [-1]     # cov(r) for current r, updated as tops placed
    r = 0
    guard = 0
    while unsched and guard < 400:
        guard += 1
        # place an AG top at this round?  (rounds must align with r)
        assert len(rounds) == r
        if rounds and (slots_done - covered_slots) >= AG_T:
            tops.append(r)
            covered_slots = slots_done
        # cov(r): rounds covered by AGs at tops <= r - COV_LAG
        c_r = -1
        for t in tops:
            if t <= r - COV_LAG:
                c_r = t - 1
        free, forced = [], {c: [] for c in range(NC)}
        for v in unsched:
            ps = set(parents[v - L].tolist())
            ok = True
            fcores = set()
            for p in ps:
                pr = round_of[p]
                if p < L:
                    continue
                if pr < 0:
                    ok = False
                    break
                if pr > c_r:
                    if pr <= r - 1:
                        fcores.add(core_of[p])
                    else:
                        ok = False
                        break
            if not ok:
                continue
            if v in core_lock:
                fcores.add(core_lock[v])
            if len(fcores) > 1:
                continue
            if len(fcores) == 1:
                forced[list(fcores)[0]].append(v)
            else:
                free.append(v)
        total = len(free) + sum(len(x) for x in forced.values())
        if total == 0:
            # nothing eligible: emit an empty round so indices stay aligned
            rounds.append(np.full((NC, 0), -1, np.int64))
            r += 1
            if r > 300:
                break
            continue
        maxk = 0
        for k in range(1, 65):
            if sum(max(0, k - len(forced[c])) for c in range(NC)) <= len(free):
                maxk = k
        k = min(maxk if maxk else 1, (total + NC - 1) // NC)
        free.sort(key=lambda v: -height[v])
        for c in forced:
            forced[c].sort(key=lambda v: -height[v])
        assign = {c: list(forced[c][:k]) for c in range(NC)}
        fi = 0
        for c in range(NC):
            while len(assign[c]) < k and fi < len(free):
                assign[c].append(free[fi])
                fi += 1

        def stats(v, c):
            f = cap = 0
            for p in parents[v - L]:
                if p < L:
                    cap += 1
                elif round_of[p] > c_r:
                    f += 1
                    cap += 1
                elif core_of[p] == c:
                    cap += 1
            return f, cap

        percore = {c: [(v,) + stats(v, c) for v in assign[c]] for c in range(NC)}
        for c in percore:
            percore[c].sort(key=lambda t: -t[1])
        deferred = []
        stable = False
        while not stable:
            stable = True
            kk = max((len(percore[c]) for c in range(NC)), default=0)
            for m in range(kk):
                col = [(c, percore[c][m]) for c in range(NC) if m < len(percore[c])]
                if not col:
                    continue
                a = max(t[1] for _, t in col)
                bad = [(c, t) for c, t in col if t[2] < a or t[1] > A_MAX]
                if not bad:
                    continue
                stable = False
                for c, t in bad:
                    if t[1] > A_MAX:
                        percore[c] = [x for x in percore[c] if x[0] != t[0]]
                        deferred.append((t[0], c))
                        continue
                    swapped = False
                    for m2 in range(len(percore[c])):
                        if m2 == m:
                            continue
                        t2 = percore[c][m2]
                        col_m = [(cc, x) for cc, x in col if cc != c]
                        col_m2 = [
                            (cc, percore[cc][m2])
                            for cc in range(NC)
                            if m2 < len(percore[cc]) and cc != c
                        ]
                        newa = max([x[1] for _, x in col_m] + [t2[1]])
                        newa2 = max([x[1] for _, x in col_m2] + [t[1]])
                        ok1 = all(x[2] >= newa for _, x in col_m) and t2[2] >= newa
                        ok2 = all(x[2] >= newa2 for _, x in col_m2) and t[2] >= newa2
                        if ok1 and ok2 and newa <= A_MAX and newa2 <= A_MAX:
                            percore[c][m], percore[c][m2] = t2, t
                            swapped = True
                            break
                    if not swapped:
                        percore[c] = [x for x in percore[c] if x[0] != t[0]]
                        deferred.append((t[0], c))
                break
        node_of = np.full((NC, k), -1, np.int64)
        for c in range(NC):
            for m, (v, f, cap) in enumerate(percore[c][:k]):
                node_of[c, m] = v
                round_of[v] = r
                core_of[v] = c
                unsched.discard(v)
        for v, c in deferred:
            core_lock[v] = c
        rounds.append(node_of)
        slots_done += k
        r += 1
    assert not unsched, "scheduler failed to place all nodes"
    return rounds, tops, round_of, core_of


# ---------------------------------------------------------------- bass build
def _build_bass(L, k_list, S, a_list, top_of_round, win_info):
    """k_list[r] = slots of round r; a_list[s] = own positions of slot s;
    top_of_round[r] = (off0, off1) if an AG is emitted at the top of round r
    (covering per-core slots [off0, off1)), else None.
    win_info unused here (host-side layout only)."""
    import concourse.bacc as bacc
    import concourse.bass as bass
    import concourse.mybir as mybir
    import concourse.tile as tile

    f16 = mybir.dt.float16
    f32 = mybir.dt.float32
    i32 = mybir.dt.int32
    f8 = mybir.dt.float8e3 if W_DTYPE == "f8e3" else mybir.dt.float16

    nc = bacc.Bacc(num_devices=NCORES, num_swdge_queues=4)
    OWN_ROWS = (L + S) * 128
    HB_ROWS = (L + 8 * S) * 128

    wbuf = nc.dram_tensor("wbuf", [S, 128, 16, 2, 128], f8, kind="ExternalInput")
    xt = nc.dram_tensor("xt", [L * 128, 64], f16, kind="ExternalInput")
    bbuf = nc.dram_tensor("bbuf", [128, 2 * S], f32, kind="ExternalInput")
    gidx = nc.dram_tensor("gidx", [1, 8 * S], i32, kind="ExternalInput")
    yout = nc.dram_tensor("yout", [S * 128, 64], f16, kind="ExternalOutput")
    own = nc.dram_tensor("own", [OWN_ROWS, 64], f16)
    hbuf = nc.dram_tensor("hbuf", [HB_ROWS, 64], f16, addr_space="Shared")
    rg = [list(range(NCORES))]

    with tile.TileContext(nc) as tc:
        with (
            tc.tile_pool(name="const", bufs=1) as constp,
            tc.tile_pool(name="w", bufs=16) as wp,
            tc.tile_pool(name="g", bufs=10) as gp,
            tc.tile_pool(name="y", bufs=8) as yp,
            tc.tile_pool(name="py", bufs=4, space="PSUM") as pyp,
        ):
            b_sb = constp.tile([128, 2 * S], f32)
            nc.sync.dma_start(b_sb[:], bbuf[:])
            gidx_sb = constp.tile([1, 8 * S], i32)
            nc.sync.dma_start(gidx_sb[:], gidx[:])
            # leaves into the local own buffer (scalar) + shared hbuf (gpsimd)
            nc.scalar.dma_start(own[0 : L * 128, :], xt[:])
            nc.gpsimd.dma_start(hbuf[0 : L * 128, :], xt[:])

            def emit_gathers(s, g_all, positions, region):
                """region: 'own' or 'hbuf'; engine: pos<4 -> sync else scalar."""
                for eng, lo, hi in ((nc.sync, 0, 4), (nc.scalar, 4, 8)):
                    use = [p for p in positions if lo <= p < hi]
                    if not use:
                        continue
                    p0, p1 = use[0], use[-1]
                    cols = gidx_sb[0:1, 8 * s + p0 : 8 * s + p1 + 1]
                    maxv = (OWN_ROWS if region == "own" else HB_ROWS) - 128
                    _, vals = nc.values_load_multi_w_load_instructions(
                        cols,
                        engines=[eng.engine],
                        min_val=0,
                        max_val=maxv,
                        skip_runtime_bounds_check=True,
                    )
                    src = own if region == "own" else hbuf
                    for p in use:
                        eng.dma_start(
                            g_all[:, p, :], src[bass.ds(vals[p - p0], 128), :]
                        )

            off = 0
            for r, k_r in enumerate(k_list):
                slots = list(range(off, off + k_r))
                # 1) weight DMAs (HWDGE, alternate engines)
                cur = []
                for i, s in enumerate(slots):
                    w_t = wp.tile([128, 16, 2, 128], f8, tag="w")
                    (nc.sync if (s % 2 == 0) else nc.scalar).dma_start(
                        w_t[:], wbuf[s]
                    )
                    cur.append((s, w_t))
                # 2) hbuf gathers (positions [a_s, 8)) - BEFORE this round's AG
                gtiles = []
                for s, w_t in cur:
                    g_all = gp.tile([128, 8, 64], f16, tag="g")
                    a_s = a_list[s]
                    emit_gathers(s, g_all, list(range(a_s, 8)), "hbuf")
                    gtiles.append((s, w_t, g_all))
                # 3) AllGather at this round's top (covers earlier rounds)
                if top_of_round[r] is not None:
                    o0, o1 = top_of_round[r]
                    nc.gpsimd.collective_compute(
                        "AllGather",
                        mybir.AluOpType.bypass,
                        replica_groups=rg,
                        ins=[own[(L + o0) * 128 : (L + o1) * 128, :]],
                        outs=[
                            hbuf[(L + 8 * o0) * 128 : (L + 8 * o1) * 128, :]
                        ],
                    )
                # 4) own gathers (positions [0, a_s)) + 5) compute
                for s, w_t, g_all in gtiles:
                    a_s = a_list[s]
                    emit_gathers(s, g_all, list(range(0, a_s)), "own")
                    # matmul order: hbuf (old) taps first, own (fresh) last
                    pos_order = list(range(a_s, 8)) + list(range(0, a_s))
                    th_order = [(p, h) for p in pos_order for h in range(2)]
                    pys = [
                        pyp.tile([128, 32], f32, tag="py", name=f"py{oh}")
                        for oh in range(2)
                    ]
                    for i, (p, h) in enumerate(th_order):
                        for oh in range(2):
                            nc.tensor.matmul(
                                pys[oh][:],
                                w_t[:, 2 * p + h, oh, :],
                                g_all[:, p, h * 32 : (h + 1) * 32],
                                start=(i == 0),
                                stop=(i == 15),
                            )
                    y16 = yp.tile([128, 2, 32], f16, tag="y16")
                    for oh in range(2):
                        bias = b_sb[:, 2 * s + oh : 2 * s + oh + 1]
                        nc.scalar.activation(
                            y16[:, oh, :],
                            pys[oh][:],
                            mybir.ActivationFunctionType.Identity,
                            bias=bias,
                            scale=float(1.0 / W_SCALE),
                        )
                    nc.gpsimd.dma_start(
                        own[(L + s) * 128 : (L + s + 1) * 128, :], y16[:]
                    )
                off += k_r

            nc.sync.dma_start(yout[:], own[L * 128 : (L + S) * 128, :])
    nc.compile()
    return nc


# ---------------------------------------------------------------- host glue
def kernel(x, W, b, parents):
    import ml_dtypes
    from concourse.bass_utils import run_bass_kernel_spmd

    x = np.ascontiguousarray(np.asarray(x), dtype=np.float32)
    W = np.ascontiguousarray(np.asarray(W), dtype=np.float32)
    b = np.ascontiguousarray(np.asarray(b), dtype=np.float32)
    parents = np.asarray(parents).astype(np.int64)

    N, C, L = x.shape
    M, O, C2, P = W.shape
    assert (N, C, O, C2, P) == (32, 256, 256, 256, 8), "kernel hardcodes these dims"

    rounds, tops, round_of, core_of = _compute_schedule(parents, L, M)
    k_list = [nd.shape[1] for nd in rounds]
    R = len(rounds)
    S = sum(k_list)
    off_of_round = np.concatenate([[0], np.cumsum(k_list)]).astype(np.int64)

    # per-core slot index of each node; global hbuf slot via AG windows
    slot_of = np.full(L + M, -1, np.int64)  # per-core slot s
    node_of_coreslot = np.full((NCORES, S), -1, np.int64)
    round_of_slot = np.zeros(S, np.int64)
    for r, nd in enumerate(rounds):
        for m in range(nd.shape[1]):
            s = off_of_round[r] + m
            round_of_slot[s] = r
            for q in range(NCORES):
                v = nd[q, m]
                if v >= 0:
                    slot_of[v] = s
                    node_of_coreslot[q, s] = v

    # AG windows: top at round t covers per-core slots [off0, off1)
    # hbuf layout (after leaves): for window j, rank-major:
    #   row of (core q, slot s) = L + 8*off0_j + q*win_j + (s - off0_j)
    top_of_round = [None] * R
    win_of_slot = np.full(S, -1, np.int64)
    wins = []
    prev = 0
    for t in tops:
        o0, o1 = prev, int(off_of_round[t])
        if o1 > o0:
            top_of_round[t] = (o0, o1)
            wins.append((o0, o1))
            win_of_slot[o0:o1] = len(wins) - 1
            prev = o1

    def hbuf_row(q, s):
        j = win_of_slot[s]
        assert j >= 0
        o0, o1 = wins[j]
        return (L + 8 * o0 + q * (o1 - o0) + (s - o0)) * 128

    # cov(r) for tap classification (must match scheduler's view)
    def cov(r):
        c = -1
        for t in tops:
            if t <= r - COV_LAG:
                c = t - 1
        return c

    # per (core, slot): tap permutation + a_s (shared across cores)
    a_list = np.zeros(S, np.int64)
    perm = np.zeros((NCORES, S, P), np.int64)  # position -> original tap
    gidx_vals = np.zeros((NCORES, S, P), np.int64)
    for s in range(S):
        r = round_of_slot[s]
        c_r = cov(r)
        # a_s = max over cores of #window parents
        amax = 0
        for q in range(NCORES):
            v = node_of_coreslot[q, s]
            if v < 0:
                continue
            nf = sum(
                1 for p in parents[v - L] if p >= L and round_of[p] > c_r
            )
            amax = max(amax, nf)
        a_list[s] = amax
        for q in range(NCORES):
            v = node_of_coreslot[q, s]
            if v < 0:
                perm[q, s] = np.arange(P)
                gidx_vals[q, s] = 0
                continue
            ps = parents[v - L]
            window, local_old, leaf, remote = [], [], [], []
            for t_i, p in enumerate(ps):
                if p < L:
                    leaf.append(t_i)
                elif round_of[p] > c_r:
                    assert core_of[p] == q and round_of[p] < r
                    window.append(t_i)
                elif core_of[p] == q:
                    local_old.append(t_i)
                else:
                    remote.append(t_i)
            own_side = window + local_old + leaf  # priority for own positions
            need = amax
            own_taps = own_side[:need]
            assert len(own_taps) == need, (
                f"slot {s} core {q}: cannot fill {need} own positions"
            )
            rest = [t_i for t_i in range(P) if t_i not in own_taps]
            # rest must be hbuf-eligible: leaf or covered computed
            for t_i in rest:
                p = ps[t_i]
                assert p < L or round_of[p] <= c_r, "window tap in hbuf position"
            order = own_taps + rest
            perm[q, s] = order
            for pos, t_i in enumerate(order):
                p = ps[t_i]
                if pos < amax:  # own region
                    if p < L:
                        gidx_vals[q, s, pos] = p * 128
                    else:
                        gidx_vals[q, s, pos] = (L + slot_of[p]) * 128
                else:  # hbuf region
                    if p < L:
                        gidx_vals[q, s, pos] = p * 128
                    else:
                        gidx_vals[q, s, pos] = hbuf_row(core_of[p], slot_of[p])
    assert a_list.max() <= A_MAX

    # ---- weight relayout: [M, o, c, p] -> [128, 16(ktile=2*pos+h), 2(oh), 128]
    # with tap permutation applied per (core, slot).
    W4 = W.transpose(0, 3, 2, 1).reshape(M, 8, 2, 128, 2, 128)
    # W4[m, tap, h(c//128), c%128, oh, o%128]
    if W_DTYPE == "f8e3":
        W4q = np.clip(W4 * W_SCALE, -15.5, 15.5).astype(ml_dtypes.float8_e3m4)
        wdt = ml_dtypes.float8_e3m4
    else:
        W4q = (W4 * W_SCALE).astype(np.float16)
        wdt = np.float16
    xt_host = np.ascontiguousarray(
        (x.transpose(2, 1, 0) / W_SCALE)
        .reshape(L, 2, 128, 32)
        .transpose(0, 2, 1, 3)
        .reshape(L * 128, 64)
        .astype(np.float16)
    )

    in_maps = []
    for q in range(NCORES):
        nodes_q = node_of_coreslot[q]
        Wq = np.zeros((S, 128, 16, 2, 128), wdt)
        bq = np.zeros((S, 2, 128), np.float32)
        for s in range(S):
            v = nodes_q[s]
            if v < 0:
                continue
            # build [128, 16, 2, 128]: ktile = 2*pos + h -> W4q[v, perm[pos], h]
            wv = W4q[v - L]  # [8, 2, 128, 2, 128]
            wp_ = wv[perm[q, s]]  # [8(pos), 2(h), 128(part), 2(oh), 128(o)]
            Wq[s] = wp_.reshape(16, 128, 2, 128).transpose(1, 0, 2, 3)
            bq[s] = (b[v - L] / W_SCALE).reshape(2, 128)
        bq2 = np.ascontiguousarray(bq.transpose(2, 0, 1).reshape(128, 2 * S))
        gq = np.ascontiguousarray(
            gidx_vals[q].reshape(1, 8 * S).astype(np.int32)
        )
        in_maps.append({"wbuf": Wq, "xt": xt_host, "bbuf": bq2, "gidx": gq})

    key = (
        L,
        tuple(k_list),
        tuple(a_list.tolist()),
        tuple(tops),
        W_DTYPE,
    )
    if key not in _BUILD_CACHE:
        import time as _time

        _t0 = _time.time()
        _BUILD_CACHE[key] = _build_bass(
            L, k_list, S, a_list.tolist(), top_of_round, wins
        )
        print(f"[kernel] bass build took {_time.time() - _t0:.1f}s", flush=True)
    nc = _BUILD_CACHE[key]

    global LAST_RUN
    LAST_RUN = (nc, in_maps)

    results = run_bass_kernel_spmd(nc, in_maps, core_ids=list(range(NCORES))).results

    out = np.zeros((N, C, L + M), np.float32)
    out[:, :, :L] = x
    for q in range(NCORES):
        yq = (
            np.asarray(results[q]["yout"])
            .astype(np.float32)
            .reshape(S, 128, 2, 32)
            .transpose(0, 3, 2, 1)
            .reshape(S, 32, 256)
        ) * W_SCALE
        for s in range(S):
            v = node_of_coreslot[q, s]
            if v >= 0:
                out[:, :, v] = yq[s]  # v is already L-based
    return out
